# revision 17
# baseline (speedup 1.0000x reference)
"""Trainium2 Bass kernel for nn_ASTPathsEncoder.

Strategy: shard the P=8192 paths across 8 NeuronCores (1024 each).
Per core: gather node encodings (compact per-core table, dma_gather),
orientation path folded into a one-hot matmul against a fused table
B = orient_emb @ proj_W @ gru_Wx (computed on device), GRU recurrence
with transposed state h^T [256, 1024] (float32r matmuls), per-step
outputs scattered to DRAM in original row order, then a duplicate-free
grouped dma_scatter_add implements segment_sum into a per-core [N] f32
accumulator. Host sums the 8 partial accumulators and concatenates the
path-sharded outputs.
"""
import sys

sys.path.insert(0, "/opt/trn_rl_repo")

import numpy as np

import concourse.bacc as bacc
import concourse.mybir as mybir
import concourse.tile as tile
from concourse.masks import make_identity

dt = mybir.dt

N, P, L, D, V = 50000, 8192, 32, 256, 16
T = 2 * L
NCORES = 8
PC = P // NCORES  # paths per core
D3 = 3 * D  # 768
ACC_SHIFT = 48  # node n lives at acc row n + ACC_SHIFT
ACC_ROWS = N + 2 * ACC_SHIFT  # 50096; rows 0..47 and 50048..50095 are pads
HI_BASE = 32768 + ACC_SHIFT - 32768  # see views below
CH = 1024  # phase-C chunk size (entries per gather/scatter instruction)
BIG = 30.0  # sigmoid(30) == 1.0 in fp32


def _ceil(a, b):
    return (a + b - 1) // b * b


def _wrap16(vals, width16):
    """Pack int list into [128, width16] int16: unwrapped[i] = w[i%16, i//16],
    replicated 8x across partition groups for the 8 Q7 cores."""
    n = len(vals)
    assert n <= width16 * 16
    w = np.full((16, width16), -1, np.int16)
    if n:
        v = np.asarray(vals, np.int64)
        assert v.min() >= 0 and v.max() < 32768, (v.min(), v.max())
        w[np.arange(n) % 16, np.arange(n) // 16] = v.astype(np.int16)
    return np.tile(w, (8, 1))


def _pass_splits(k):
    out = []
    b = 0
    while b < k:
        w = min(512, k - b)
        out.append((b, w))
        b += w
    return out


# ----------------------------------------------------------------------------
# Host-side preparation
# ----------------------------------------------------------------------------

def _prepare(ast_nodes, path_idx, seq_len, child_place, vertical_dir):
    """Compute the shared schedule + per-core input arrays."""
    cores = []
    for c in range(NCORES):
        sl = slice(c * PC, (c + 1) * PC)
        cores.append(dict(
            idx=path_idx[sl].astype(np.int64),
            lens=seq_len[sl].astype(np.int64),
            cp=child_place[sl].astype(np.int64),
            vd=vertical_dir[sl].astype(np.int64),
        ))

    # sorted-by-length column order per core; global step widths k_t
    for cd in cores:
        order = np.argsort(-cd["lens"], kind="stable")
        cd["order"] = order
        cd["slens"] = cd["lens"][order]
    a = np.zeros((NCORES, T), np.int64)
    for ci, cd in enumerate(cores):
        for t in range(T):
            a[ci, t] = int((2 * cd["slens"] > t).sum())
    k_t = [min(PC, int(_ceil(int(a[:, t].max()), 128))) for t in range(T)]

    # compact node tables
    for cd in cores:
        glob = np.unique(cd["idx"])
        cd["glob"] = glob
        cd["lidx"] = np.searchsorted(glob, cd["idx"])  # [PC, L]
    n_max = max(len(cd["glob"]) for cd in cores)
    n_max = _ceil(max(n_max, 128), 128)
    assert n_max <= 32768

    # x-gather index packing (even steps), widths k_t[2l]
    gx_off = []
    off = 0
    for l in range(L):
        gx_off.append(off)
        off += k_t[2 * l] // 16
    W_gx = max(off, 1)

    # phase C grouping (duplicate-free rounds, split lo/hi destination halves)
    # entry: src row p*L+l in nodes_out, dst acc row idx+ACC_SHIFT
    for cd in cores:
        pp, ll = np.nonzero(np.arange(L)[None, :] < cd["lens"][:, None])
        src = pp * L + ll
        dst = cd["idx"][pp, ll] + ACC_SHIFT
        o2 = np.argsort(dst, kind="stable")
        src, dst = src[o2], dst[o2]
        # occurrence rank within each dst run
        if len(dst):
            first = np.r_[True, dst[1:] != dst[:-1]]
            gidx = np.cumsum(first) - 1
            starts = np.nonzero(first)[0]
            rank = np.arange(len(dst)) - starts[gidx]
        else:
            rank = np.zeros(0, np.int64)
        cd["pc_entries"] = (src, dst, rank)
    G = max(1, max((int(cd["pc_entries"][2].max()) + 1 if len(cd["pc_entries"][2]) else 1)
                   for cd in cores))
    # chunk counts per (round, half)
    n_chunks = np.zeros((G, 2), np.int64)
    for cd in cores:
        src, dst, rank = cd["pc_entries"]
        for r in range(G):
            m = rank == r
            lo = int((dst[m] < 32768).sum())
            hi = int(m.sum()) - lo
            n_chunks[r, 0] = max(n_chunks[r, 0], (lo + CH - 1) // CH)
            n_chunks[r, 1] = max(n_chunks[r, 1], (hi + CH - 1) // CH)
    chunk_list = []  # (round, half)
    for r in range(G):
        for h in range(2):
            chunk_list += [(r, h)] * int(n_chunks[r, h])
    NCHUNK = len(chunk_list)

    schedule = dict(k_t=k_t, n_max=n_max, gx_off=gx_off, W_gx=W_gx,
                    G=G, n_chunks=n_chunks.tolist(), chunk_list=chunk_list,
                    NCHUNK=NCHUNK)

    # per-core device inputs
    iota32 = (np.arange(32, dtype=np.float32) % 16).reshape(32, 1)
    sel = np.zeros((2, 32), np.float32)
    sel[0, :16] = 1.0
    sel[1, 16:] = 1.0
    in_maps = []
    for cd in cores:
        order = cd["order"]
        slens = cd["slens"]
        # compact table
        comp = np.zeros((n_max, D), np.float32)
        comp[: len(cd["glob"])] = ast_nodes[cd["glob"]]
        # x-gather indices
        gxw = np.full((16, W_gx), 0, np.int16)
        for l in range(L):
            k = k_t[2 * l]
            if k == 0:
                continue
            vals = np.zeros(k, np.int64)
            nact = min(k, PC)
            vals[:nact] = cd["lidx"][order[:nact], l]
            blk = _wrap16(vals, k // 16)
            gxw[:, gx_off[l]: gx_off[l] + k // 16] = blk[:16]
        gxw = np.tile(gxw, (8, 1))
        # h-out scatter rows: step t -> row order[i]*L + t//2
        hout = np.zeros((128, T * (PC // 16)), np.int16)
        for t in range(T):
            rows = order * L + (t // 2)
            hout[:, t * (PC // 16): (t + 1) * (PC // 16)] = _wrap16(rows, PC // 16)
        # inactive mask [T, PC] (sorted order)
        tt = np.arange(T)[:, None]
        inact = (tt >= 2 * slens[None, :]).astype(np.float32)
        cpf = cd["cp"][order].T.astype(np.float32).copy()  # [L, PC]
        vdf = cd["vd"][order].T.astype(np.float32).copy()
        # phase C chunk index arrays
        src, dst, rank = cd["pc_entries"]
        pcg = np.zeros((128, max(NCHUNK, 1) * (CH // 16)), np.int16)
        pcs = np.zeros((128, max(NCHUNK, 1) * (CH // 16)), np.int16)
        ptr = {}
        for r in range(G):
            for h in range(2):
                m = (rank == r) & ((dst < 32768) if h == 0 else (dst >= 32768))
                ptr[(r, h)] = [src[m], dst[m], 0]
        for ci, (r, h) in enumerate(chunk_list):
            s_, d_, used = ptr[(r, h)]
            take = min(CH, len(s_) - used)
            gsrc = np.zeros(CH, np.int64)
            gdst = np.zeros(CH, np.int64)
            # pads: src row 0; dst -> trash rows (dup races allowed there)
            gdst[:] = (np.arange(CH) % ACC_SHIFT) if h == 0 else (
                32720 + (np.arange(CH) % ACC_SHIFT))
            if take > 0:
                gsrc[:take] = s_[used: used + take]
                gdst[:take] = (d_[used: used + take] if h == 0
                               else d_[used: used + take] - 17328)
                ptr[(r, h)][2] = used + take
            pcg[:, ci * (CH // 16): (ci + 1) * (CH // 16)] = _wrap16(gsrc, CH // 16)
            pcs[:, ci * (CH // 16): (ci + 1) * (CH // 16)] = _wrap16(gdst, CH // 16)

        in_maps.append({
            "comp": comp,
            "gxidx": gxw,
            "houtidx": hout,
            "pcgidx": pcg,
            "pcsidx": pcs,
            "inact": inact,
            "cpf": cpf,
            "vdf": vdf,
            "iota32": iota32,
            "sel": sel,
        })
    return schedule, in_maps


# ----------------------------------------------------------------------------
# Device program
# ----------------------------------------------------------------------------

def _build(schedule, weights):
    k_t = schedule["k_t"]
    n_max = schedule["n_max"]
    gx_off = schedule["gx_off"]
    W_gx = schedule["W_gx"]
    NCHUNK = schedule["NCHUNK"]

    nc = bacc.Bacc("TRN2", target_bir_lowering=False, debug=False,
                   num_devices=NCORES)

    # inputs
    comp = nc.dram_tensor("comp", [n_max, D], dt.float32, kind="ExternalInput")
    gxidx = nc.dram_tensor("gxidx", [128, W_gx], dt.int16, kind="ExternalInput")
    houtidx = nc.dram_tensor("houtidx", [128, T * (PC // 16)], dt.int16,
                             kind="ExternalInput")
    pcgidx = nc.dram_tensor("pcgidx", [128, max(NCHUNK, 1) * (CH // 16)],
                            dt.int16, kind="ExternalInput")
    pcsidx = nc.dram_tensor("pcsidx", [128, max(NCHUNK, 1) * (CH // 16)],
                            dt.int16, kind="ExternalInput")
    inact_in = nc.dram_tensor("inact", [T, PC], dt.float32, kind="ExternalInput")
    cpf_in = nc.dram_tensor("cpf", [L, PC], dt.float32, kind="ExternalInput")
    vdf_in = nc.dram_tensor("vdf", [L, PC], dt.float32, kind="ExternalInput")
    iota32_in = nc.dram_tensor("iota32", [32, 1], dt.float32, kind="ExternalInput")
    sel_in = nc.dram_tensor("sel", [2, 32], dt.float32, kind="ExternalInput")
    oe_in = nc.dram_tensor("orient_emb", [V, D], dt.float32, kind="ExternalInput")
    pw_in = nc.dram_tensor("proj_W", [2 * D, D], dt.float32, kind="ExternalInput")
    pb_in = nc.dram_tensor("proj_b", [D], dt.float32, kind="ExternalInput")
    wx_in = nc.dram_tensor("gru_Wx", [D, D3], dt.float32, kind="ExternalInput")
    wh_in = nc.dram_tensor("gru_Wh", [D, D3], dt.float32, kind="ExternalInput")
    bx_in = nc.dram_tensor("gru_bx", [D3], dt.float32, kind="ExternalInput")
    bh_in = nc.dram_tensor("gru_bh", [D3], dt.float32, kind="ExternalInput")

    # outputs (pre-zeroed by the runner)
    nodes_out = nc.dram_tensor("nodes_out", [PC * L, D], dt.float32,
                               kind="ExternalOutput")
    orient_out = nc.dram_tensor("orient_out", [PC * L, D], dt.float32,
                                kind="ExternalOutput")
    acc = nc.dram_tensor("acc", [ACC_ROWS, D], dt.float32, kind="ExternalOutput")

    f32, f32r = dt.float32, dt.float32r
    SIG = mybir.ActivationFunctionType.Sigmoid
    TANH = mybir.ActivationFunctionType.Tanh
    MUL = mybir.AluOpType.mult
    ADD = mybir.AluOpType.add
    SUB = mybir.AluOpType.subtract
    ISEQ = mybir.AluOpType.is_equal

    with tile.TileContext(nc) as tc:
        with (
            tc.tile_pool(name="const", bufs=1) as cpool,
            tc.tile_pool(name="work", bufs=2) as wpool,
            tc.tile_pool(name="rows", bufs=1) as rpool,
            tc.tile_pool(name="psum", bufs=1, space="PSUM") as pp,
        ):
            # --- persistent constants -------------------------------------
            ident = cpool.tile([128, 128], f32)
            make_identity(nc, ident[:])

            whr = cpool.tile([128, 2, D3], f32r, tag="whr")
            wxr = cpool.tile([128, 2, D3], f32r, tag="wxr")
            wtmp = cpool.tile([128, 2, D3], f32, tag="wtmp")
            for src, dstt in ((wh_in, whr), (wx_in, wxr)):
                nc.sync.dma_start(out=wtmp[:, 0, :], in_=src[0:128, :])
                nc.sync.dma_start(out=wtmp[:, 1, :], in_=src[128:256, :])
                nc.gpsimd.tensor_copy(dstt[:], wtmp[:])

            # proj_W chunks [128, 4, 256] (W1 = ch 0,1; W2 = ch 2,3)
            pwr = cpool.tile([128, 4, D], f32r, tag="pwr")
            pwtmp = cpool.tile([128, 4, D], f32, tag="pwtmp")
            for ch in range(4):
                nc.sync.dma_start(out=pwtmp[:, ch, :],
                                  in_=pw_in[ch * 128:(ch + 1) * 128, :])
            nc.gpsimd.tensor_copy(pwr[:], pwtmp[:])

            # oe^T [128, 2, 16] via strided DMA, then f32r
            oeT = cpool.tile([128, 2, V], f32r, tag="oeT")
            oeTtmp = cpool.tile([128, 2, V], f32, tag="oeTtmp")
            for kk in range(2):
                nc.sync.dma_start(
                    out=oeTtmp[:, kk, :],
                    in_=oe_in[:, kk * 128:(kk + 1) * 128].rearrange("v p -> p v"))
            nc.gpsimd.tensor_copy(oeT[:], oeTtmp[:])
            # pb^T [128, 2]
            pbT = cpool.tile([128, 2, 1], f32r, tag="pbT")
            pbTtmp = cpool.tile([128, 2, 1], f32, tag="pbTtmp")
            for kk in range(2):
                nc.sync.dma_start(out=pbTtmp[:, kk, 0],
                                  in_=pb_in[kk * 128:(kk + 1) * 128, None].rearrange("p o -> p o"))
            nc.gpsimd.tensor_copy(pbT[:], pbTtmp[:])

            # ACT bias columns [128, 6]: m0-1 r:+(bx+bh), m2-3 z:-(bx+bh), m4-5 n:+bx
            bxc = cpool.tile([128, 6], f32, tag="bxc")
            bhc = cpool.tile([128, 6], f32, tag="bhc")
            nc.sync.dma_start(out=bxc[:], in_=bx_in[:].rearrange("(m p) -> p m", p=128))
            nc.sync.dma_start(out=bhc[:], in_=bh_in[:].rearrange("(m p) -> p m", p=128))
            biases = cpool.tile([128, 6], f32, tag="biases")
            nc.vector.tensor_tensor(out=biases[:, 0:4], in0=bxc[:, 0:4],
                                    in1=bhc[:, 0:4], op=ADD)
            nc.vector.tensor_scalar_mul(biases[:, 2:4], biases[:, 2:4], -1.0)
            nc.vector.tensor_copy(biases[:, 4:6], bxc[:, 4:6])

            # bh_n as K=1 lhsT row [1, 256]
            bhn = cpool.tile([1, D], f32r, tag="bhn")
            bhn_tmp = cpool.tile([1, D], f32, tag="bhn_tmp")
            nc.sync.dma_start(out=bhn_tmp[:], in_=bh_in[None, 512:768])
            nc.gpsimd.tensor_copy(bhn[:], bhn_tmp[:])

            fstage1 = cpool.tile([1, PC], f32, tag="fstage1")
            fstage2 = cpool.tile([1, PC], f32, tag="fstage2")
            nc.vector.memset(fstage1[:], BIG)
            nc.vector.memset(fstage2[:], 1.0)
            zbig = cpool.tile([1, D], f32r, tag="zbig")
            nc.gpsimd.tensor_copy(zbig[:], fstage1[:, :D])
            onesrow = cpool.tile([1, PC], f32r, tag="onesrow")
            nc.gpsimd.tensor_copy(onesrow[:], fstage2[:])

            iota32 = cpool.tile([32, 1], f32, tag="iota32")
            nc.sync.dma_start(out=iota32[:], in_=iota32_in[:])
            sel_tmp = cpool.tile([2, 32], f32, tag="sel_tmp")
            nc.sync.dma_start(out=sel_tmp[:], in_=sel_in[:])
            selr = cpool.tile([2, 32], f32r, tag="selr")
            nc.gpsimd.tensor_copy(selr[:], sel_tmp[:])


            gxidx_t = cpool.tile([128, W_gx], dt.int16, tag="gxidx")
            nc.sync.dma_start(out=gxidx_t[:], in_=gxidx[:])
            houtidx_t = cpool.tile([128, T * (PC // 16)], dt.int16, tag="houtidx")
            nc.sync.dma_start(out=houtidx_t[:], in_=houtidx[:])

            # --- fused orientation table Boh [33, 768] ---------------------
            # rows 0:16 = B1 = (oe @ W1) @ Wx ; rows 16:32 = B2 ; row 32 = pb @ Wx
            boh = cpool.tile([33, D3], f32r, tag="boh")
            a1t = cpool.tile([128, 2, V], f32r, tag="a1t")
            a2t = cpool.tile([128, 2, V], f32r, tag="a2t")
            for w12, adst in ((0, a1t), (2, a2t)):
                for m in range(2):
                    ps = pp.tile([128, V], f32, tag="g0")
                    for kk in range(2):
                        nc.tensor.matmul(
                            ps[:], lhsT=pwr[:, w12 + kk, m * 128:(m + 1) * 128],
                            rhs=oeT[:, kk, :], start=(kk == 0), stop=(kk == 1))
                    nc.vector.tensor_copy(adst[:, m, :], ps[:])
            for rowbase, lhsT_src in ((0, a1t), (16, a2t)):
                for ns in range(2):
                    nsl = slice(ns * 384, (ns + 1) * 384)
                    ps = pp.tile([16, 384], f32, tag="g1")
                    for kk in range(2):
                        nc.tensor.matmul(ps[:], lhsT=lhsT_src[:, kk, :],
                                         rhs=wxr[:, kk, nsl],
                                         start=(kk == 0), stop=(kk == 1))
                    if rowbase == 0:
                        nc.vector.tensor_copy(boh[0:16, nsl], ps[:])
                    else:
                        btmp = cpool.tile([16, 384], f32r, tag="btmp")
                        nc.vector.tensor_copy(btmp[:], ps[:])
                        nc.sync.dma_start(out=boh[16:32, nsl], in_=btmp[:])
            for ns in range(2):
                nsl = slice(ns * 384, (ns + 1) * 384)
                ps = pp.tile([1, 384], f32, tag="g2")
                for kk in range(2):
                    nc.tensor.matmul(ps[:], lhsT=pbT[:, kk, :], rhs=wxr[:, kk, nsl],
                                     start=(kk == 0), stop=(kk == 1))
                nc.vector.tensor_copy(boh[32:33, nsl], ps[:])

            # --- state ------------------------------------------------------
            h0 = cpool.tile([128, PC], f32, tag="h0")
            h1 = cpool.tile([128, PC], f32, tag="h1")
            hr0 = cpool.tile([128, PC], f32r, tag="hr0")
            hr1 = cpool.tile([128, PC], f32r, tag="hr1")
            for tl in (h0, h1):
                nc.vector.memset(tl[:], 0.0)
            nc.gpsimd.tensor_copy(hr0[:], h0[:])
            nc.gpsimd.tensor_copy(hr1[:], h1[:])
            H = (h0, h1)
            HR = (hr0, hr1)

            # --- recurrence ---------------------------------------------------
            for t in range(T):
                l = t // 2
                even = t % 2 == 0
                k = k_t[t]
                splits = _pass_splits(k)

                xT = None
                ohsb = None
                inrow_t = rpool.tile([1, PC], f32, tag="inrow_t")
                nc.sync.dma_start(out=inrow_t[:], in_=inact_in[t:t + 1, :])
                inrow = rpool.tile([1, PC], f32r, tag="inrow")
                nc.gpsimd.tensor_copy(inrow[:], inrow_t[:])
                if k > 0 and even:
                    gx = wpool.tile([128, max(k // 128, 1), D], f32, tag="gx")
                    nc.gpsimd.dma_gather(
                        out_ap=gx[:], in_ap=comp[:],
                        idxs_ap=gxidx_t[:, gx_off[l]: gx_off[l] + k // 16],
                        num_idxs=k, num_idxs_reg=k, elem_size=D)
                    xT = wpool.tile([128, 2, PC], f32r, tag="xT")
                    for (b, w) in splits:
                        for dh in range(2):
                            ps = pp.tile([128, 512], f32, tag="xt")
                            for j in range(w // 128):
                                nc.tensor.transpose(
                                    ps[:, j * 128:(j + 1) * 128],
                                    gx[:, b // 128 + j, dh * 128:(dh + 1) * 128],
                                    ident[:])
                            nc.vector.tensor_copy(xT[:, dh, b:b + w], ps[:, :w])
                elif k > 0:
                    cvf = rpool.tile([2, PC], f32, tag="cvf")
                    nc.sync.dma_start(out=cvf[0:1, :], in_=cpf_in[l:l + 1, :])
                    nc.sync.dma_start(out=cvf[1:2, :], in_=vdf_in[l:l + 1, :])
                    cvr = rpool.tile([2, PC], f32r, tag="cvr")
                    nc.gpsimd.tensor_copy(cvr[:], cvf[:])
                    ohsb = wpool.tile([33, PC], f32r, tag="ohsb")
                    nc.vector.tensor_copy(ohsb[32:33, :], onesrow[:])
                    for (b, w) in splits:
                        ps_oh = pp.tile([32, 512], f32, tag="ohp")
                        nc.tensor.matmul(ps_oh[:, :w], lhsT=selr[:],
                                         rhs=cvr[:, b:b + w],
                                         start=True, stop=True)
                        nc.vector.tensor_tensor(
                            out=ohsb[0:32, b:b + w], in0=ps_oh[:, :w],
                            in1=iota32[:].to_broadcast([32, w]), op=ISEQ)

                for (b, w) in splits:
                    bsl = slice(b, b + w)
                    # gate preactivations r (m=0,1), z (m=2,3)
                    rt, wt = [], []
                    for m in range(4):
                        msl = slice(m * 128, (m + 1) * 128)
                        ps = pp.tile([128, 512], f32, tag=f"g{m}")
                        chunks = []
                        if even:
                            chunks += [(wxr[:, kk, msl], xT[:, kk, bsl])
                                       for kk in range(2)]
                        else:
                            chunks += [(boh[:, msl], ohsb[:, bsl])]
                        if t > 0:
                            chunks += [(whr[:, kk, msl], HR[kk][:, bsl])
                                       for kk in range(2)]
                        if m >= 2:
                            chunks += [(zbig[:, msl.start - 256:msl.stop - 256],
                                        inrow[:, bsl])]
                        for ci_, (lh, rh) in enumerate(chunks):
                            nc.tensor.matmul(ps[:, :w], lhsT=lh, rhs=rh,
                                             start=(ci_ == 0),
                                             stop=(ci_ == len(chunks) - 1))
                        dst = wpool.tile([128, 512], f32, tag=f"rw{m}")
                        if m < 2:
                            nc.scalar.activation(out=dst[:, :w], in_=ps[:, :w],
                                                 func=SIG, bias=biases[:, m:m + 1])
                        else:
                            nc.scalar.activation(out=dst[:, :w], in_=ps[:, :w],
                                                 func=SIG, scale=-1.0,
                                                 bias=biases[:, m:m + 1])
                        (rt if m < 2 else wt).append(dst)

                    # n gates (m=4,5) + h update
                    for m in (4, 5):
                        dh = m - 4
                        msl = slice(m * 128, (m + 1) * 128)
                        hn = pp.tile([128, 512], f32, tag=f"g{dh}")
                        chunks = []
                        if t > 0:
                            chunks += [(whr[:, kk, msl], HR[kk][:, bsl])
                                       for kk in range(2)]
                        chunks += [(bhn[:, dh * 128:(dh + 1) * 128],
                                    onesrow[:, bsl])]
                        for ci_, (lh, rh) in enumerate(chunks):
                            nc.tensor.matmul(hn[:, :w], lhsT=lh, rhs=rh,
                                             start=(ci_ == 0),
                                             stop=(ci_ == len(chunks) - 1))
                        xn = pp.tile([128, 512], f32, tag=f"g{dh + 2}")
                        if even:
                            chunks = [(wxr[:, kk, msl], xT[:, kk, bsl])
                                      for kk in range(2)]
                        else:
                            chunks = [(boh[:, msl], ohsb[:, bsl])]
                        for ci_, (lh, rh) in enumerate(chunks):
                            nc.tensor.matmul(xn[:, :w], lhsT=lh, rhs=rh,
                                             start=(ci_ == 0),
                                             stop=(ci_ == len(chunks) - 1))
                        t1 = wpool.tile([128, 512], f32, tag=f"t1_{dh}")
                        nc.vector.tensor_tensor(out=t1[:, :w], in0=rt[dh][:, :w],
                                                in1=hn[:, :w], op=MUL)
                        nc.vector.tensor_tensor(out=t1[:, :w], in0=t1[:, :w],
                                                in1=xn[:, :w], op=ADD)
                        nt = wpool.tile([128, 512], f32, tag=f"nt{dh}")
                        nc.scalar.activation(out=nt[:, :w], in_=t1[:, :w],
                                             func=TANH, bias=biases[:, m:m + 1])
                        am = wpool.tile([128, 512], f32, tag=f"am{dh}")
                        nc.vector.tensor_tensor(out=am[:, :w], in0=nt[:, :w],
                                                in1=H[dh][:, bsl], op=SUB)
                        nc.vector.tensor_tensor(out=am[:, :w], in0=wt[dh][:, :w],
                                                in1=am[:, :w], op=MUL)
                        nc.vector.tensor_tensor(out=H[dh][:, bsl],
                                                in0=H[dh][:, bsl],
                                                in1=am[:, :w], op=ADD)
                    for dh in range(2):
                        nc.gpsimd.tensor_copy(HR[dh][:, bsl], H[dh][:, bsl])

                # h-out: transpose full width, scatter to original rows
                hout = wpool.tile([128, PC // 128, D], f32, tag="hout")
                for jj in range(PC // 256):
                    ps = pp.tile([128, 512], f32, tag="hop")
                    for j2 in range(2):
                        j = jj * 2 + j2
                        nc.tensor.transpose(
                            ps[:, j2 * 256: j2 * 256 + 128],
                            h0[:, j * 128:(j + 1) * 128], ident[:])
                        nc.tensor.transpose(
                            ps[:, j2 * 256 + 128: j2 * 256 + 256],
                            h1[:, j * 128:(j + 1) * 128], ident[:])
                    nc.scalar.copy(
                        out=hout[:, jj * 2: jj * 2 + 2, :].rearrange("p a b -> p (a b)"),
                        in_=ps[:])
                dst_t = nodes_out if even else orient_out
                nc.gpsimd.dma_scatter_add(
                    out_ap=dst_t[:], in_ap=hout[:],
                    idxs_ap=houtidx_t[:, t * (PC // 16): (t + 1) * (PC // 16)],
                    num_idxs=PC, num_idxs_reg=PC, elem_size=D)

            # --- phase C: segment-sum scatter ------------------------------
            pcg_t = cpool.tile([128, max(NCHUNK, 1) * (CH // 16)], dt.int16,
                               tag="pcg")
            pcs_t = cpool.tile([128, max(NCHUNK, 1) * (CH // 16)], dt.int16,
                               tag="pcs")
            nc.sync.dma_start(out=pcg_t[:], in_=pcgidx[:])
            nc.sync.dma_start(out=pcs_t[:], in_=pcsidx[:])
            for ci_, (r, hf) in enumerate(schedule["chunk_list"]):
                gt = wpool.tile([128, CH // 128, D], f32, tag="gx")
                isl = slice(ci_ * (CH // 16), (ci_ + 1) * (CH // 16))
                nc.gpsimd.dma_gather(
                    out_ap=gt[:], in_ap=nodes_out[:], idxs_ap=pcg_t[:, isl],
                    num_idxs=CH, num_idxs_reg=CH, elem_size=D)
                out_view = acc[0:32768, :] if hf == 0 else acc[17328:ACC_ROWS, :]
                nc.gpsimd.dma_scatter_add(
                    out_ap=out_view, in_ap=gt[:], idxs_ap=pcs_t[:, isl],
                    num_idxs=CH, num_idxs_reg=CH, elem_size=D)

    nc.compile()
    return nc


# ----------------------------------------------------------------------------
# Entry point
# ----------------------------------------------------------------------------

_CACHE = {}


def _get_program(schedule, weights):
    key = (tuple(schedule["k_t"]), schedule["n_max"], schedule["W_gx"],
           schedule["G"], tuple(map(tuple, schedule["n_chunks"])))
    if key not in _CACHE:
        _CACHE[key] = _build(schedule, weights)
    return _CACHE[key]


def kernel(ast_nodes_encodings, path_node_indices, seq_lengths, child_place,
           vertical_dir, orient_emb, proj_W, proj_b,
           gru_Wx, gru_Wh, gru_bx, gru_bh, _run=None):
    ast_nodes_encodings = np.asarray(ast_nodes_encodings, np.float32)
    orient_emb = np.asarray(orient_emb, np.float32)
    proj_W = np.asarray(proj_W, np.float32)
    proj_b = np.asarray(proj_b, np.float32)
    gru_Wx = np.asarray(gru_Wx, np.float32)
    gru_Wh = np.asarray(gru_Wh, np.float32)
    gru_bx = np.asarray(gru_bx, np.float32)
    gru_bh = np.asarray(gru_bh, np.float32)

    schedule, in_maps = _prepare(ast_nodes_encodings,
                                 np.asarray(path_node_indices),
                                 np.asarray(seq_lengths),
                                 np.asarray(child_place),
                                 np.asarray(vertical_dir))
    weights = dict(orient_emb=orient_emb, proj_W=proj_W,
                   proj_b=proj_b.reshape(D),
                   gru_Wx=gru_Wx, gru_Wh=gru_Wh,
                   gru_bx=gru_bx.reshape(D3), gru_bh=gru_bh.reshape(D3))
    for m in in_maps:
        m.update(weights)

    nc = _get_program(schedule, weights)

    if _run is not None:
        results = _run(nc, in_maps)
    else:
        from concourse.bass_utils import run_bass_kernel_spmd
        results = run_bass_kernel_spmd(
            nc, in_maps, core_ids=list(range(NCORES))).results

    new_nodes = np.zeros((N, D), np.float32)
    paths_nodes_enc = np.empty((P, L, D), np.float32)
    orient_enc = np.empty((P, L, D), np.float32)
    for c in range(NCORES):
        new_nodes += results[c]["acc"][ACC_SHIFT: ACC_SHIFT + N]
        paths_nodes_enc[c * PC:(c + 1) * PC] = \
            results[c]["nodes_out"].reshape(PC, L, D)
        orient_enc[c * PC:(c + 1) * PC] = \
            results[c]["orient_out"].reshape(PC, L, D)
    return new_nodes, paths_nodes_enc, orient_enc
